# revision 47
# baseline (speedup 1.0000x reference)
"""GPT-2 block (B=2, T=2048, C=768, H=12) on 8 Trainium2 NeuronCores.

Sharding: data-parallel over batch (2) x 4-way query-tile split per batch.
Each core computes K/V for its full batch (avoids on-chip collectives,
whose latency floor exceeds the redundant compute) and runs attention +
MLP for 4 of the 16 query tiles, interleaved {g, 7-g, 8+g, 15-g} so the
causal-attention work is identical across cores.

The SPMD program is uniform across cores: per-core differences are pushed
into the data via a k-tile permutation of the sequence (each core's query
tiles sit at fixed positions {3,7,11,15}; every tile's causal prefix is
placed before it) plus per-core causal masks.

Layouts: activations enter matmuls feature-major (xnT [C,T]) so QKV needs
no transposes; attention scores are computed transposed (S^T [k,q]) so
exp(S^T) is directly the stationary operand of the A*V matmul, and a ones
column appended to V produces the softmax denominator in the same matmul.

Precision: QKV weights and activations are fp8 e4m3 with DoubleRow
matmuls (2 K-rows/cycle, K-tiles of 256); weights are pre-scaled by 32
on the host (folds into the exp() scale and the V ones column). The
MLP and attention projection stay bf16 - fp8 fc1/fc2 was measured at
rel-err 0.020 alone, which does not fit the 2e-2 budget on top of the
attention path's 0.008. x streams in as bf16 (f32 only for the
residual copy of the core's own tokens).

LayerNorm gains/biases are folded into the following weight matrices on
the host; rstd is computed with a Newton-Raphson rsqrt on the vector
engine (variance is ~1 for LN inputs here) so the scalar engine needs
only two activation-table loads (exp, gelu) for the whole kernel.
K/Q biases ride per-feature bias adds during PSUM evacuation, split
between the scalar (ACT identity+bias) and vector (tensor_scalar)
engines; big late-phase weights are DMA'd from the scalar stream
mid-phase-1 so they don't contend with the x tiles.
"""

import sys

sys.path.insert(0, "/opt/trn_rl_repo")

import numpy as np
import ml_dtypes

import bass_rust
import concourse.bass as bass
import concourse.bacc as bacc
import concourse.tile as tile
from concourse import mybir
from concourse.vector_clock import ScopedClock

BF16 = ml_dtypes.bfloat16
F32 = mybir.dt.float32
BF = mybir.dt.bfloat16
F8 = mybir.dt.float8e4
NP_F8 = mybir.dt.np(F8)

B, T, C, H = 2, 2048, 768, 12
HD = C // H  # 64
DFF = 4 * C  # 3072
TT = T // 128  # 16 token tiles
CT = C // 128  # 6 feature tiles
KT = C // 256  # 3 DoubleRow k-tiles over C
KT2 = DFF // 256  # 12 DoubleRow k-tiles over DFF
FT = DFF // 128  # 24
QPOS = (3, 7, 11, 15)  # fixed positions of this core's query tiles
NQ = 512  # queries per core
WS = 32.0  # fp8 weight pre-scale
WARM = 80  # PE p-state warm-up matmuls
AF = mybir.ActivationFunctionType
ALU = mybir.AluOpType
DR = mybir.MatmulPerfMode.DoubleRow

# ---------------------------------------------------------------------------
# Tile exit-drain fix: the final SP drain carries one wait per live logical
# processor, but TRN2 ISA instructions hold at most 1 embedded sync wait in
# this toolchain. Split the waits across a chain of SP drains.
# ---------------------------------------------------------------------------
_MAX_WAITS = 1


def _drain_and_barrier(self, tick_clock, wait_clock):
    drain_inst = self.nc.sync.drain()
    wait_clock.add_sem_waits(
        drain_inst.ins, ScopedClock({None: tick_clock.global_clock})
    )
    si = drain_inst.ins.sync_info
    if si is not None and len(si.on_wait) > _MAX_WAITS:
        waits = list(si.on_wait)
        drain_inst.ins.sync_info = bass_rust.SyncInfo(
            on_wait=waits[:_MAX_WAITS], on_update=list(si.on_update)
        )
        rest = waits[_MAX_WAITS:]
        for i in range(0, len(rest), _MAX_WAITS):
            extra = self.nc.sync.drain()
            extra.ins.sync_info = bass_rust.SyncInfo(
                on_wait=rest[i : i + _MAX_WAITS], on_update=[]
            )
    self.nc.all_engine_barrier()
    assert self.sems is not None
    popped = self.nc._tile_sem_poison_stack.pop()
    assert popped is self._sem_poison
    self.nc.clear_and_free_semaphores(list(self.sems.allocated().values()))
    self.nc.all_engine_barrier()


tile.TileContext._drain_and_barrier = _drain_and_barrier


# ---------------------------------------------------------------------------
# Per-core sharding layout (host side)
# ---------------------------------------------------------------------------
def core_layout(g):
    """For group index g (0..3): (qtiles sorted, perm) with the core's query
    tiles at positions QPOS and every tile's causal prefix placed before it."""
    qtiles = sorted([g, 7 - g, 8 + g, 15 - g])
    posmap = dict(zip(QPOS, qtiles))
    rest = iter([t for t in range(TT) if t not in qtiles])
    perm = [posmap[p] if p in posmap else next(rest) for p in range(TT)]
    for j, a in enumerate(qtiles):
        assert set(range(a + 1)) <= set(perm[: QPOS[j] + 1]), (g, j, perm)
    return qtiles, perm


def core_masks(qtiles, perm):
    """masks[kp] = causal mask of k-position kp against query tile j=kp//4
    (the first in-suffix block - across all core layouts the only block
    that is ever not all-ones)."""
    masks = np.zeros((TT, 128, 128), dtype=BF16)
    for kp in range(TT):
        tk = perm[kp] * 128 + np.arange(128)[:, None]
        a = qtiles[kp // 4]
        tq = a * 128 + np.arange(128)[None, :]
        masks[kp] = (tk <= tq).astype(BF16)
    return masks


def pack_dr(W):
    """[K, N] fp32 -> DoubleRow-paired fp8 [K/256, 128, 2, N], pre-scaled.
    Logical k = 256*kt + 128*r + p."""
    K, N = W.shape
    Wp = (np.asarray(W, np.float32) * WS).reshape(K // 256, 2, 128, N)
    return np.ascontiguousarray(Wp.transpose(0, 2, 1, 3)).astype(NP_F8)


# ---------------------------------------------------------------------------
# The Bass program (identical for all 8 cores)
# ---------------------------------------------------------------------------
def build_program():
    nc = bacc.Bacc("TRN2")

    d_x = nc.dram_tensor("x_perm", [T, C], BF, kind="ExternalInput")
    d_xob = nc.dram_tensor("x_own_b", [NQ, C], F32, kind="ExternalInput")
    d_masks = nc.dram_tensor("masks", [TT, 128, 128], BF, kind="ExternalInput")
    d_wq = nc.dram_tensor("wq", [KT, 128, 2, C], F8, kind="ExternalInput")
    d_wk = nc.dram_tensor("wk", [KT, 128, 2, C], F8, kind="ExternalInput")
    d_wv = nc.dram_tensor("wv", [KT, 128, 2, C], F8, kind="ExternalInput")
    d_wp = nc.dram_tensor("wp", [C, C], BF, kind="ExternalInput")
    d_wfc = nc.dram_tensor("wfc", [C, DFF], BF, kind="ExternalInput")
    d_wfc2 = nc.dram_tensor("wfc2", [DFF, C], BF, kind="ExternalInput")
    # [bq bk] each [128, CT] (scaled by WS)
    d_cn = nc.dram_tensor("consts2", [128, 2 * CT], F32, kind="ExternalInput")
    # bfc2 broadcast [128, C]
    d_cb = nc.dram_tensor("constsb", [128, C], F32, kind="ExternalInput")
    d_bfc = nc.dram_tensor("bfc2d", [128, FT], F32, kind="ExternalInput")
    d_ident = nc.dram_tensor("ident", [128, 128], BF, kind="ExternalInput")
    d_out = nc.dram_tensor("out", [NQ, C], F32, kind="ExternalOutput")

    with tile.TileContext(nc) as tc:
        _body(nc, tc, locals())
    nc.compile()
    return nc


def _nr_rsqrt(nc, pool, v_ap, n, iters=3):
    """rstd [128, n] = 1/sqrt(v + eps) via linear seed + NR iterations on the
    vector engine (v is a LayerNorm variance, tightly around 1)."""
    y = pool.tile([128, n], F32, tag="nr_y", name="nr_y")
    # seed: y0 = 1.5 - 0.5 v   (v==1 -> exact)
    nc.vector.tensor_scalar(
        out=y[:], in0=v_ap, scalar1=-0.5, scalar2=1.5, op0=ALU.mult, op1=ALU.add
    )
    t = pool.tile([128, n], F32, tag="nr_t", name="nr_t")
    u = pool.tile([128, n], F32, tag="nr_u", name="nr_u")
    for _ in range(iters):
        # t = y*y ; u = (-0.5 t) * v ; y = (u + 1.5) * y
        nc.vector.scalar_tensor_tensor(
            out=t[:], in0=y[:], scalar=1.0, in1=y[:], op0=ALU.mult, op1=ALU.mult
        )
        nc.vector.scalar_tensor_tensor(
            out=u[:], in0=t[:], scalar=-0.5, in1=v_ap, op0=ALU.mult, op1=ALU.mult
        )
        nc.vector.scalar_tensor_tensor(
            out=y[:], in0=u[:], scalar=1.5, in1=y[:], op0=ALU.add, op1=ALU.mult
        )
    return y


def _body(nc, tc, d):
    def pool(name, **kw):
        return tc.tile_pool(name=name, **kw)

    with (
        pool("const", bufs=1) as constp,
        pool("persist", bufs=1) as pers,
        pool("small", bufs=6) as small,
    ):
        # ---- constants ---------------------------------------------------
        ident = constp.tile([128, 128], BF)
        nc.sync.dma_start(ident[:], d["d_ident"][:])
        cn = constp.tile([128, 2, CT], F32)
        nc.sync.dma_start(cn[:], d["d_cn"][:].rearrange("p (a b) -> p a b", b=CT))
        bq, bk = cn[:, 0, :], cn[:, 1, :]
        bfc2_bc = constp.tile([128, C], F32)
        nc.sync.dma_start(bfc2_bc[:], d["d_cb"][:])
        bfcc = constp.tile([128, FT], F32)
        nc.sync.dma_start(bfcc[:], d["d_bfc"][:])
        zc = constp.tile([128, H, HD], BF)
        nc.vector.memset(zc[:], 0.0)

        # ---- persistent tensors -----------------------------------------
        wp_t = pers.tile([128, CT, C], BF, tag="wp", name="wp_t")
        wp = [wp_t[:, t, :] for t in range(CT)]
        wfc_t = pers.tile([128, CT, DFF], BF, tag="wfc", name="wfc_t")
        wfc = [wfc_t[:, t, :] for t in range(CT)]
        wfc2A = pers.tile([128, FT // 2, C], BF, tag="wfc2A", name="wfc2A")
        xo_t = pers.tile([128, 4, C], F32, tag="xo", name="xo_t")
        x_own = [xo_t[:, j, :] for j in range(4)]
        qT = [pers.tile([128, NQ], BF, tag=f"qT{t}", name=f"qT{t}") for t in range(CT)]
        yT = [pers.tile([128, NQ], BF, tag=f"yT{t}", name=f"yT{t}") for t in range(CT)]

        with pool("attn_sb", bufs=1) as attnp:
            kT = [attnp.tile([128, T], BF, tag=f"kT{t}", name=f"kT{t}") for t in range(CT)]
            V = [attnp.tile([128, H * (HD + 32)], BF, tag=f"V{t}", name=f"V{t}") for t in range(TT)]
            masks_t = attnp.tile([128, TT, 128], BF, tag="masks", name="masks_t")
            masks = [masks_t[:, t, :] for t in range(TT)]

            # ======== phase 1: LN1 + transpose to feature-major fp8 ========
            with (
                pool("ph1", bufs=1) as ph1p,
                pool("ph1s", bufs=6) as ph1s,
                pool("ph1n", bufs=4) as ph1n,
                pool("wqkv", bufs=1) as wp_,
            ):
                xnT8 = [ph1p.tile([128, 2, T], F8, tag=f"xnT{t}", name=f"xnT{t}") for t in range(KT)]
                xnTq8 = [ph1p.tile([128, 2, NQ], F8, tag=f"xnTq{t}", name=f"xnTq{t}") for t in range(KT)]
                wq8_t = wp_.tile([128, KT, 2, C], F8, tag="wq8", name="wq8")
                wk8_t = wp_.tile([128, KT, 2, C], F8, tag="wk8", name="wk8")
                wv8_t = wp_.tile([128, KT, 2, C], F8, tag="wv8", name="wv8")
                wq8 = [wq8_t[:, t, :, :] for t in range(KT)]
                wk8 = [wk8_t[:, t, :, :] for t in range(KT)]
                wv8 = [wv8_t[:, t, :, :] for t in range(KT)]
                xts = {}

                def x_dma(tt, eng):
                    xts[tt] = ph1s.tile([128, C], BF, tag="xt", name="xt")
                    eng.dma_start(xts[tt][:], d["d_x"][tt * 128 : (tt + 1) * 128, :])

                # x tiles split across the sync and gpsimd queues (halves the
                # per-queue issue seriality; buffer-reuse waits on one queue
                # can't starve the other); QKV weights early on sync; big
                # late-phase weights from the scalar stream mid-phase-1.
                for tt in range(8):
                    x_dma(tt, nc.sync)
                for tt in range(8, 16):
                    x_dma(tt, nc.gpsimd)
                nc.gpsimd.dma_start(xo_t[:], d["d_xob"][:].rearrange("(t p) c -> p t c", p=128))
                nc.sync.dma_start(wk8_t[:], d["d_wk"][:].rearrange("k p r c -> p k r c"))
                nc.sync.dma_start(wq8_t[:], d["d_wq"][:].rearrange("k p r c -> p k r c"))
                nc.sync.dma_start(wv8_t[:], d["d_wv"][:].rearrange("k p r c -> p k r c"))
                nc.sync.dma_start(masks_t[:], d["d_masks"][:].rearrange("k p c -> p k c"))

                with (
                    pool("ph1t", bufs=2, space="PSUM") as ph1t,
                    pool("ph2k", bufs=2, space="PSUM") as ph2k,
                ):
                    # PE warm-up: ramp the p-state while the first LayerNorm
                    # group is still in flight
                    warm = ph2k.tile([128, 512], F32, tag="pqk", name="warm")
                    for _ in range(WARM):
                        nc.tensor.matmul(warm[:, 0:128], ident[:], ident[:])
                    for ttg in range(4):
                        if ttg == 1:
                            # big late-phase weights: issued from the scalar
                            # stream once the early x tiles are in flight
                            nc.scalar.dma_start(wp_t[:], d["d_wp"][:].rearrange("(t p) c -> p t c", p=128))
                            nc.scalar.dma_start(wfc_t[:], d["d_wfc"][:].rearrange("(t p) c -> p t c", p=128))
                            nc.scalar.dma_start(
                                wfc2A[:],
                                d["d_wfc2"][: FT // 2 * 128, :].rearrange("(t p) c -> p t c", p=128),
                            )
                        mvb = small.tile([128, 4, 2], F32, tag="mvb", name="mvb")
                        for ti in range(4):
                            tt = ttg * 4 + ti
                            stats = small.tile([128, 3, 6], F32, tag="bnstats", name="bnstats")
                            xg = xts[tt][:].rearrange("p (a b) -> p a b", b=256)
                            for a in range(3):
                                nc.vector.bn_stats(out=stats[:, a, :], in_=xg[:, a, :])
                            nc.vector.bn_aggr(out=mvb[:, ti, :], in_=stats[:])
                        # rstd for the 4 tiles of this group in one NR batch
                        # (eps is negligible against var ~ 1)
                        rstd4 = _nr_rsqrt(nc, small, mvb[:, :, 1], 4, iters=2)
                        ptb = ph1t.tile([128, CT, 512], BF, tag="ptb", name="ptb")
                        for ti in range(4):
                            tt = ttg * 4 + ti
                            xn = ph1n.tile([128, C], BF, tag="xn", name="xn")
                            nc.vector.tensor_scalar(
                                out=xn[:], in0=xts[tt][:],
                                scalar1=mvb[:, ti, 0:1], scalar2=rstd4[:, ti : ti + 1],
                                op0=ALU.subtract, op1=ALU.mult,
                            )
                            for ct in range(CT):
                                nc.tensor.transpose(
                                    ptb[:, ct, ti * 128 : (ti + 1) * 128],
                                    xn[:, ct * 128 : (ct + 1) * 128], ident[:],
                                )
                        # fp8 conversion: pure casts on the scalar engine,
                        # one ACT per DoubleRow pair
                        for kt in range(KT):
                            nc.scalar.activation(
                                out=xnT8[kt][:, :, ttg * 512 : (ttg + 1) * 512],
                                in_=ptb[:, 2 * kt : 2 * kt + 2, :], func=AF.Identity,
                            )
                            # own q-tile of this group sits at ti == 3
                            nc.scalar.activation(
                                out=xnTq8[kt][:, :, ttg * 128 : (ttg + 1) * 128],
                                in_=ptb[:, 2 * kt : 2 * kt + 2, 384:512],
                                func=AF.Identity,
                            )
                        # kT chunk ttg depends only on this token-group
                        for f in range(CT):
                            ps = ph2k.tile([128, 512], F32, tag="pqk", name="pk")
                            for kt in range(KT):
                                nc.tensor.matmul(
                                    ps[:], wk8[kt][:, :, f * 128 : (f + 1) * 128],
                                    xnT8[kt][:, :, ttg * 512 : (ttg + 1) * 512],
                                    start=(kt == 0), stop=(kt == KT - 1),
                                    perf_mode=DR,
                                )
                            dstk = kT[f][:, ttg * 512 : (ttg + 1) * 512]
                            if f < 4:
                                nc.scalar.activation(
                                    out=dstk, in_=ps[:], func=AF.Identity,
                                    bias=bk[:, f : f + 1],
                                )
                            else:
                                nc.vector.tensor_scalar(
                                    out=dstk, in0=ps[:], scalar1=bk[:, f : f + 1],
                                    scalar2=None, op0=ALU.add,
                                )

                # ======== phase 2: V (with bias-row), Q^T (fp8 DoubleRow) ====
                with (
                    pool("ph2ps", bufs=2, space="PSUM") as ph2ps,
                    pool("ph2pv", bufs=2, space="PSUM") as ph2pv,
                ):
                    def v_tiles(lo_t, hi_t):
                        for tt in range(lo_t, hi_t):
                            pv = ph2pv.tile([128, C], F32, tag="pv", name="pv")
                            for lo, hi in ((0, 512), (512, 768)):
                                for kt in range(KT):
                                    nc.tensor.matmul(
                                        pv[:, lo:hi],
                                        xnT8[kt][:, :, tt * 128 : (tt + 1) * 128],
                                        wv8[kt][:, :, lo:hi],
                                        start=(kt == 0), stop=(kt == KT - 1),
                                        perf_mode=DR,
                                    )
                            vt = V[tt][:].rearrange("p (h e) -> p h e", e=HD + 32)
                            pvh = pv[:].rearrange("p (h e) -> p h e", e=HD)
                            nc.gpsimd.memset(vt[:, :, HD : HD + 32], WS)
                            nc.vector.tensor_copy(vt[:, :, 0:HD], pvh[:])
                    # V tiles 0..11 (needed from attention kp=0/4/8)
                    v_tiles(0, 12)
                    # qT[f] [128, 512] = (Wq[:, f].T @ xnT[own q tiles]) + bq
                    for f in range(CT):
                        ps = ph2ps.tile([128, NQ], F32, tag="pq", name="pq")
                        for kt in range(KT):
                            nc.tensor.matmul(
                                ps[:], wq8[kt][:, :, f * 128 : (f + 1) * 128],
                                xnTq8[kt][:], start=(kt == 0), stop=(kt == KT - 1),
                                perf_mode=DR,
                            )
                        if f % 2 == 0:
                            nc.scalar.activation(
                                out=qT[f][:], in_=ps[:], func=AF.Identity,
                                bias=bq[:, f : f + 1],
                            )
                        else:
                            nc.vector.tensor_scalar(
                                out=qT[f][:], in0=ps[:], scalar1=bq[:, f : f + 1],
                                scalar2=None, op0=ALU.add,
                            )
                    # V tiles 12..15 are only read at attention kp>=12
                    v_tiles(12, 16)

            # ======== phase 3: attention (bf16, 4 heads in flight) ========
            # exp is batched over head PAIRS (one ACT op per pair) since ACT
            # runs 1x with a ~352-cycle fixed cost per instruction.
            with (
                pool("ph3", bufs=8) as ph3s,
                pool("ph3ps", bufs=2, space="PSUM") as ph3ps,
                pool("ph3pa", bufs=1, space="PSUM") as ph3pa,
            ):
                for hg in range(H // 4):
                    hs = [hg * 4 + i for i in range(4)]
                    pavs = {
                        h: ph3pa.tile(
                            [128, NQ], F32, tag=f"pav{h % 4}", name=f"pav{h % 4}"
                        )
                        for h in hs
                    }
                    for kp in range(TT):
                        cs = 128 * (kp // 4)
                        psbs = {}
                        for pi in range(2):
                            hA, hB = hs[2 * pi], hs[2 * pi + 1]
                            ps2 = ph3ps.tile([128, 2, NQ], F32, tag="ps2", name="ps2")
                            for r, h in ((0, hA), (1, hB)):
                                ro = (h % 2) * 64
                                nc.tensor.matmul(
                                    ps2[:, r, cs:NQ],
                                    kT[h // 2][ro : ro + 64, kp * 128 : (kp + 1) * 128],
                                    qT[h // 2][ro : ro + 64, cs:NQ],
                                )
                            p_sb = ph3s.tile([128, 2, NQ], BF, tag="p_sb", name="p_sb")
                            # q,k both carry the 32x fp8 weight scale
                            nc.scalar.activation(
                                out=p_sb[:, :, cs:NQ], in_=ps2[:, :, cs:NQ],
                                func=AF.Exp, scale=0.125 / (WS * WS),
                            )
                            # only the first in-suffix 128-col block is ever
                            # not all-ones (across every core layout)
                            for r, h in ((0, hA), (1, hB)):
                                nc.vector.tensor_mul(
                                    p_sb[:, r, cs : cs + 128],
                                    p_sb[:, r, cs : cs + 128], masks[kp][:],
                                )
                                psbs[h] = (p_sb, r)
                        for h in hs:
                            p_sb, r = psbs[h]
                            nc.tensor.matmul(
                                pavs[h][0 : HD + 32, cs:NQ],
                                V[kp][:, h * (HD + 32) : (h + 1) * (HD + 32)],
                                p_sb[:, r, cs:NQ],
                                start=(kp == 0), stop=(kp == TT - 1),
                                skip_group_check=True,
                            )
                    # gather the 4 heads' denominators and invert them in a
                    # single wide fast-reciprocal
                    denall = ph3s.tile([128, NQ], F32, tag="denall", name="denall")
                    for i, h in enumerate(hs):
                        nc.vector.tensor_copy(
                            denall[32 * i : 32 * (i + 1), :],
                            pavs[h][HD : HD + 32, :],
                        )
                    rball = ph3s.tile([128, NQ], F32, tag="rball", name="rball")
                    nc.vector.reciprocal_approx_fast(out=rball[:], in_=denall[:])
                    for i, h in enumerate(hs):
                        ro = (h % 2) * 64
                        for half in range(2):
                            nc.vector.tensor_tensor(
                                out=yT[h // 2][ro + 32 * half : ro + 32 * (half + 1), :],
                                in0=pavs[h][32 * half : 32 * (half + 1), :],
                                in1=rball[32 * i : 32 * (i + 1), :], op=ALU.mult,
                            )

        # ======== phase 4: proj + residual + LN2 ========
        with pool("mlp_sb", bufs=1) as mlpp:
            x2_t = mlpp.tile([128, 4, C], F32, tag="x2", name="x2_t")
            x2 = [x2_t[:, j, :] for j in range(4)]
            x2b_t = mlpp.tile([128, 4, C], F32, tag="x2b", name="x2b_t")
            x2b = [x2b_t[:, j, :] for j in range(4)]
            xn2_t = mlpp.tile([128, CT, NQ], BF, tag="xn2T", name="xn2_t")
            xn2T = [xn2_t[:, t, :] for t in range(CT)]
            hT_t = mlpp.tile([128, FT, NQ], BF, tag="hT", name="hT_t")
            hT = [hT_t[:, t, :] for t in range(FT)]
            wfc2B = mlpp.tile([128, FT // 2, C], BF, tag="wfc2B", name="wfc2B")
            nc.sync.dma_start(
                wfc2B[:],
                d["d_wfc2"][FT // 2 * 128 :, :].rearrange("(t p) c -> p t c", p=128),
            )
            wfc2 = [wfc2A[:, t, :] for t in range(FT // 2)] + [
                wfc2B[:, t, :] for t in range(FT // 2)
            ]
            with (
                pool("mlp1s", bufs=3) as m1s,
                pool("ph4p", bufs=2, space="PSUM") as ph4p,
                pool("ph4t", bufs=4, space="PSUM") as ph4t,
            ):
                for qt in range(4):
                    pp = ph4p.tile([128, C], F32, tag="pp", name="pp")
                    for lo, hi in ((0, 512), (512, 768)):
                        for ct in range(CT):
                            nc.tensor.matmul(
                                pp[:, lo:hi],
                                yT[ct][:, qt * 128 : (qt + 1) * 128],
                                wp[ct][:, lo:hi],
                                start=(ct == 0), stop=(ct == CT - 1),
                            )
                    nc.vector.tensor_add(x2[qt][:], pp[:], x_own[qt][:])
                    stats = small.tile([128, 3, 6], F32, tag="bnstats", name="bnstats")
                    xg = x2[qt][:].rearrange("p (a b) -> p a b", b=256)
                    for a in range(3):
                        nc.vector.bn_stats(out=stats[:, a, :], in_=xg[:, a, :])
                    mv = small.tile([128, 2], F32, tag="bnaggr", name="bnaggr")
                    nc.vector.bn_aggr(out=mv[:], in_=stats[:])
                    rstd = _nr_rsqrt(nc, small, mv[:, 1:2], 1, iters=2)
                    xn2 = m1s.tile([128, C], BF, tag="xn2", name="xn2")
                    nc.vector.tensor_scalar(
                        out=xn2[:], in0=x2[qt][:], scalar1=mv[:, 0:1], scalar2=rstd[:],
                        op0=ALU.subtract, op1=ALU.mult,
                    )
                    for ct in range(CT):
                        pt = ph4t.tile([128, 128], BF, tag="pt4", name="pt4")
                        nc.tensor.transpose(
                            pt[:], xn2[:, ct * 128 : (ct + 1) * 128], ident[:]
                        )
                        nc.scalar.activation(
                            out=xn2_t[:, ct, qt * 128 : (qt + 1) * 128],
                            in_=pt[:], func=AF.Identity,
                        )

            # ======== phase 5: fc -> hT directly (feature-major out), gelu
            # bias is per-partition; fc2 for query tiles 0/1 streams along as
            # each hT f-tile lands, so fc2 overlaps fc instead of following it
            with pool("po01", bufs=1, space="PSUM") as po01p:
                poA = [
                    po01p.tile([128, C], F32, tag=f"poA{q}", name=f"poA{q}")
                    for q in range(2)
                ]
                with pool("ph5p", bufs=2, space="PSUM") as ph5p:
                    for f in range(FT):
                        ph_ = ph5p.tile([128, 512], F32, tag="ph5", name="ph5")
                        for ct in range(CT):
                            nc.tensor.matmul(
                                ph_[:],
                                wfc[ct][:, f * 128 : (f + 1) * 128],
                                xn2T[ct][:],
                                start=(ct == 0), stop=(ct == CT - 1),
                            )
                        nc.scalar.activation(
                            out=hT[f][:], in_=ph_[:],
                            func=AF.Gelu_apprx_tanh, bias=bfcc[:, f : f + 1],
                        )
                        if f < 4:
                            nc.vector.tensor_add(x2b[f][:], x2[f][:], bfc2_bc[:])
                        for qi in range(2):
                            for lo, hi in ((0, 512), (512, 768)):
                                nc.tensor.matmul(
                                    poA[qi][:, lo:hi],
                                    hT[f][:, qi * 128 : (qi + 1) * 128],
                                    wfc2[f][:, lo:hi],
                                    start=(f == 0), stop=(f == FT - 1),
                                    skip_group_check=True,
                                )

                # ======== phase 7: fc2 for query tiles 2/3 + residual + out ==
                with (
                    pool("mlp2s", bufs=3) as m2s,
                    pool("ph7p", bufs=2, space="PSUM") as ph7p,
                ):
                    for qt in range(4):
                        if qt < 2:
                            po = poA[qt]
                        else:
                            po = ph7p.tile([128, C], F32, tag="po", name="po")
                            for lo, hi in ((0, 512), (512, 768)):
                                for kt in range(FT):
                                    nc.tensor.matmul(
                                        po[:, lo:hi],
                                        hT[kt][:, qt * 128 : (qt + 1) * 128],
                                        wfc2[kt][:, lo:hi],
                                        start=(kt == 0), stop=(kt == FT - 1),
                                    )
                        ot = m2s.tile([128, C], F32, tag="ot", name="ot")
                        nc.vector.tensor_add(ot[:], po[:], x2b[qt][:])
                        nc.sync.dma_start(
                            d["d_out"][qt * 128 : (qt + 1) * 128, :], ot[:]
                        )


# ---------------------------------------------------------------------------
# Host-side wrapper
# ---------------------------------------------------------------------------
_PROGRAM = None


def _get_program():
    global _PROGRAM
    if _PROGRAM is None:
        _PROGRAM = build_program()
    return _PROGRAM


def make_in_maps(x, ln1_g, ln1_b, W_attn, b_attn, W_proj, b_proj,
                 ln2_g, ln2_b, W_fc, b_fc, W_fc2, b_fc2):
    x = np.asarray(x, np.float32)
    # fold LN gains/biases into the downstream weights (host-side, free)
    g1 = np.asarray(ln1_g, np.float32)
    b1 = np.asarray(ln1_b, np.float32)
    g2 = np.asarray(ln2_g, np.float32)
    b2 = np.asarray(ln2_b, np.float32)
    Wa = np.asarray(W_attn, np.float32)
    Wag = Wa * g1[:, None]
    ba = np.asarray(b_attn, np.float32) + b1 @ Wa
    Wf = np.asarray(W_fc, np.float32)
    Wfg = Wf * g2[:, None]
    bf = np.asarray(b_fc, np.float32) + b2 @ Wf

    shared = {
        "wq": pack_dr(Wag[:, 0:C]),
        "wk": pack_dr(Wag[:, C : 2 * C]),
        "wv": pack_dr(Wag[:, 2 * C : 3 * C]),
        "wp": np.asarray(W_proj, BF16),
        "wfc": np.asarray(Wfg, BF16),
        "wfc2": np.asarray(W_fc2, BF16),
        # [bq bk] ride the 32x weight scale
        "consts2": np.concatenate([
            (ba[0:C] * WS).reshape(CT, 128).T,
            (ba[C : 2 * C] * WS).reshape(CT, 128).T,
        ], axis=1).copy(),
        "constsb": np.broadcast_to(
            np.asarray(b_fc2, np.float32), (128, C)).copy(),
        "bfc2d": np.ascontiguousarray(bf.reshape(FT, 128).T),
        "ident": np.eye(128, dtype=BF16),
    }
    bp = np.asarray(b_proj, np.float32)
    in_maps, layouts = [], []
    for core in range(8):
        b, g = core // 4, core % 4
        qtiles, perm = core_layout(g)
        idx = np.concatenate([np.arange(t * 128, (t + 1) * 128) for t in perm])
        own = np.concatenate([np.arange(t * 128, (t + 1) * 128) for t in qtiles])
        m = dict(shared)
        m["x_perm"] = np.ascontiguousarray(x[b][idx]).astype(BF16)
        m["x_own_b"] = np.ascontiguousarray(x[b][own] + bp)
        m["masks"] = core_masks(qtiles, perm)
        in_maps.append(m)
        layouts.append((b, own))
    return in_maps, layouts


def unshard(results, layouts):
    out = np.empty((B, T, C), np.float32)
    for r, (b, own) in zip(results, layouts):
        out[b][own] = r["out"]
    return out


def kernel(**inputs):
    from concourse.bass_utils import run_bass_kernel_spmd

    nc = _get_program()
    in_maps, layouts = make_in_maps(**inputs)
    res = run_bass_kernel_spmd(nc, in_maps, core_ids=list(range(8)))
    return unshard(res.results, layouts)


# revision 48
# speedup vs baseline: 1.1651x; 1.1651x over previous
"""GPT-2 block (B=2, T=2048, C=768, H=12) on 8 Trainium2 NeuronCores.

Sharding: data-parallel over batch (2) x 4-way query-tile split per batch.
Each core computes K/V for its full batch (avoids on-chip collectives,
whose latency floor exceeds the redundant compute) and runs attention +
MLP for 4 of the 16 query tiles, interleaved {g, 7-g, 8+g, 15-g} so the
causal-attention work is identical across cores.

The SPMD program is uniform across cores: per-core differences are pushed
into the data via a k-tile permutation of the sequence (each core's query
tiles sit at fixed positions {3,7,11,15}; every tile's causal prefix is
placed before it) plus per-core causal masks.

Layouts: activations enter matmuls feature-major (xnT [C,T]) so QKV needs
no transposes; attention scores are computed transposed (S^T [k,q]) so
exp(S^T) is directly the stationary operand of the A*V matmul, and a ones
column appended to V produces the softmax denominator in the same matmul.

Precision: QKV weights and activations are fp8 e4m3 with DoubleRow
matmuls (2 K-rows/cycle, K-tiles of 256); weights are pre-scaled by 32
on the host (folds into the exp() scale and the V ones column). The
MLP and attention projection stay bf16 - fp8 fc1/fc2 was measured at
rel-err 0.020 alone, which does not fit the 2e-2 budget on top of the
attention path's 0.008. x streams in as bf16 (f32 only for the
residual copy of the core's own tokens).

LayerNorm gains/biases are folded into the following weight matrices on
the host; rstd is computed with a Newton-Raphson rsqrt on the vector
engine (variance is ~1 for LN inputs here) so the scalar engine needs
only two activation-table loads (exp, gelu) for the whole kernel.
K/Q biases ride per-feature bias adds during PSUM evacuation, split
between the scalar (ACT identity+bias) and vector (tensor_scalar)
engines; big late-phase weights are DMA'd from the scalar stream
mid-phase-1 so they don't contend with the x tiles.
"""

import sys

sys.path.insert(0, "/opt/trn_rl_repo")

import numpy as np
import ml_dtypes

import bass_rust
import concourse.bass as bass
import concourse.bacc as bacc
import concourse.tile as tile
from concourse import mybir
from concourse.vector_clock import ScopedClock

BF16 = ml_dtypes.bfloat16
F32 = mybir.dt.float32
BF = mybir.dt.bfloat16
F8 = mybir.dt.float8e4
NP_F8 = mybir.dt.np(F8)

B, T, C, H = 2, 2048, 768, 12
HD = C // H  # 64
DFF = 4 * C  # 3072
TT = T // 128  # 16 token tiles
CT = C // 128  # 6 feature tiles
KT = C // 256  # 3 DoubleRow k-tiles over C
KT2 = DFF // 256  # 12 DoubleRow k-tiles over DFF
FT = DFF // 128  # 24
QPOS = (3, 7, 11, 15)  # fixed positions of this core's query tiles
NQ = 512  # queries per core
WS = 32.0  # fp8 weight pre-scale
WARM = 80  # PE p-state warm-up matmuls
AF = mybir.ActivationFunctionType
ALU = mybir.AluOpType
DR = mybir.MatmulPerfMode.DoubleRow

# ---------------------------------------------------------------------------
# Tile exit-drain fix: the final SP drain carries one wait per live logical
# processor, but TRN2 ISA instructions hold at most 1 embedded sync wait in
# this toolchain. Split the waits across a chain of SP drains.
# ---------------------------------------------------------------------------
_MAX_WAITS = 1


def _drain_and_barrier(self, tick_clock, wait_clock):
    drain_inst = self.nc.sync.drain()
    wait_clock.add_sem_waits(
        drain_inst.ins, ScopedClock({None: tick_clock.global_clock})
    )
    si = drain_inst.ins.sync_info
    if si is not None and len(si.on_wait) > _MAX_WAITS:
        waits = list(si.on_wait)
        drain_inst.ins.sync_info = bass_rust.SyncInfo(
            on_wait=waits[:_MAX_WAITS], on_update=list(si.on_update)
        )
        rest = waits[_MAX_WAITS:]
        for i in range(0, len(rest), _MAX_WAITS):
            extra = self.nc.sync.drain()
            extra.ins.sync_info = bass_rust.SyncInfo(
                on_wait=rest[i : i + _MAX_WAITS], on_update=[]
            )
    self.nc.all_engine_barrier()
    assert self.sems is not None
    popped = self.nc._tile_sem_poison_stack.pop()
    assert popped is self._sem_poison
    self.nc.clear_and_free_semaphores(list(self.sems.allocated().values()))
    self.nc.all_engine_barrier()


tile.TileContext._drain_and_barrier = _drain_and_barrier


# ---------------------------------------------------------------------------
# Per-core sharding layout (host side)
# ---------------------------------------------------------------------------
def core_layout(g):
    """For group index g (0..3): (qtiles sorted, perm) with the core's query
    tiles at positions QPOS and every tile's causal prefix placed before it."""
    qtiles = sorted([g, 7 - g, 8 + g, 15 - g])
    posmap = dict(zip(QPOS, qtiles))
    rest = iter([t for t in range(TT) if t not in qtiles])
    perm = [posmap[p] if p in posmap else next(rest) for p in range(TT)]
    for j, a in enumerate(qtiles):
        assert set(range(a + 1)) <= set(perm[: QPOS[j] + 1]), (g, j, perm)
    return qtiles, perm


def core_masks(qtiles, perm):
    """masks[kp] = causal mask of k-position kp against query tile j=kp//4
    (the first in-suffix block - across all core layouts the only block
    that is ever not all-ones)."""
    masks = np.zeros((TT, 128, 128), dtype=BF16)
    for kp in range(TT):
        tk = perm[kp] * 128 + np.arange(128)[:, None]
        a = qtiles[kp // 4]
        tq = a * 128 + np.arange(128)[None, :]
        masks[kp] = (tk <= tq).astype(BF16)
    return masks


def pack_dr(W):
    """[K, N] fp32 -> DoubleRow-paired fp8 [K/256, 128, 2, N], pre-scaled.
    Logical k = 256*kt + 128*r + p."""
    K, N = W.shape
    Wp = (np.asarray(W, np.float32) * WS).reshape(K // 256, 2, 128, N)
    return np.ascontiguousarray(Wp.transpose(0, 2, 1, 3)).astype(NP_F8)


# ---------------------------------------------------------------------------
# The Bass program (identical for all 8 cores)
# ---------------------------------------------------------------------------
def build_program():
    nc = bacc.Bacc("TRN2")

    d_x = nc.dram_tensor("x_perm", [T, C], BF, kind="ExternalInput")
    d_xob = nc.dram_tensor("x_own_b", [NQ, C], F32, kind="ExternalInput")
    d_masks = nc.dram_tensor("masks", [TT, 128, 128], BF, kind="ExternalInput")
    d_wq = nc.dram_tensor("wq", [KT, 128, 2, C], F8, kind="ExternalInput")
    d_wk = nc.dram_tensor("wk", [KT, 128, 2, C], F8, kind="ExternalInput")
    d_wv = nc.dram_tensor("wv", [KT, 128, 2, C], F8, kind="ExternalInput")
    d_wp = nc.dram_tensor("wp", [C, C], BF, kind="ExternalInput")
    d_wfc = nc.dram_tensor("wfc", [C, DFF], BF, kind="ExternalInput")
    d_wfc2 = nc.dram_tensor("wfc2", [DFF, C], BF, kind="ExternalInput")
    # [bq bk] each [128, CT] (scaled by WS)
    d_cn = nc.dram_tensor("consts2", [128, 2 * CT], F32, kind="ExternalInput")
    # bfc2 broadcast [128, C]
    d_cb = nc.dram_tensor("constsb", [128, C], F32, kind="ExternalInput")
    d_bfc = nc.dram_tensor("bfc2d", [128, FT], F32, kind="ExternalInput")
    d_ident = nc.dram_tensor("ident", [128, 128], BF, kind="ExternalInput")
    d_out = nc.dram_tensor("out", [NQ, C], F32, kind="ExternalOutput")

    with tile.TileContext(nc) as tc:
        _body(nc, tc, locals())
    nc.compile()
    return nc


def _nr_rsqrt(nc, pool, v_ap, n, iters=3):
    """rstd [128, n] = 1/sqrt(v + eps) via linear seed + NR iterations on the
    vector engine (v is a LayerNorm variance, tightly around 1)."""
    y = pool.tile([128, n], F32, tag="nr_y", name="nr_y")
    # seed: y0 = 1.5 - 0.5 v   (v==1 -> exact)
    nc.vector.tensor_scalar(
        out=y[:], in0=v_ap, scalar1=-0.5, scalar2=1.5, op0=ALU.mult, op1=ALU.add
    )
    t = pool.tile([128, n], F32, tag="nr_t", name="nr_t")
    u = pool.tile([128, n], F32, tag="nr_u", name="nr_u")
    for _ in range(iters):
        # t = y*y ; u = (-0.5 t) * v ; y = (u + 1.5) * y
        nc.vector.scalar_tensor_tensor(
            out=t[:], in0=y[:], scalar=1.0, in1=y[:], op0=ALU.mult, op1=ALU.mult
        )
        nc.vector.scalar_tensor_tensor(
            out=u[:], in0=t[:], scalar=-0.5, in1=v_ap, op0=ALU.mult, op1=ALU.mult
        )
        nc.vector.scalar_tensor_tensor(
            out=y[:], in0=u[:], scalar=1.5, in1=y[:], op0=ALU.add, op1=ALU.mult
        )
    return y


def _body(nc, tc, d):
    def pool(name, **kw):
        return tc.tile_pool(name=name, **kw)

    with (
        pool("const", bufs=1) as constp,
        pool("persist", bufs=1) as pers,
        pool("small", bufs=6) as small,
    ):
        # ---- constants ---------------------------------------------------
        ident = constp.tile([128, 128], BF)
        nc.sync.dma_start(ident[:], d["d_ident"][:])
        cn = constp.tile([128, 2, CT], F32)
        nc.sync.dma_start(cn[:], d["d_cn"][:].rearrange("p (a b) -> p a b", b=CT))
        bq, bk = cn[:, 0, :], cn[:, 1, :]
        bfc2_bc = constp.tile([128, C], F32)
        nc.sync.dma_start(bfc2_bc[:], d["d_cb"][:])
        bfcc = constp.tile([128, FT], F32)
        nc.sync.dma_start(bfcc[:], d["d_bfc"][:])
        zc = constp.tile([128, H, HD], BF)
        nc.vector.memset(zc[:], 0.0)

        # ---- persistent tensors -----------------------------------------
        wp_t = pers.tile([128, CT, C], BF, tag="wp", name="wp_t")
        wp = [wp_t[:, t, :] for t in range(CT)]
        wfc_t = pers.tile([128, CT, DFF], BF, tag="wfc", name="wfc_t")
        wfc = [wfc_t[:, t, :] for t in range(CT)]
        wfc2A = pers.tile([128, FT // 2, C], BF, tag="wfc2A", name="wfc2A")
        xo_t = pers.tile([128, 4, C], F32, tag="xo", name="xo_t")
        x_own = [xo_t[:, j, :] for j in range(4)]
        qT = [pers.tile([128, NQ], BF, tag=f"qT{t}", name=f"qT{t}") for t in range(CT)]
        yT = [pers.tile([128, NQ], BF, tag=f"yT{t}", name=f"yT{t}") for t in range(CT)]

        with pool("attn_sb", bufs=1) as attnp:
            kT = [attnp.tile([128, T], BF, tag=f"kT{t}", name=f"kT{t}") for t in range(CT)]
            V = [attnp.tile([128, H * (HD + 32)], BF, tag=f"V{t}", name=f"V{t}") for t in range(TT)]
            masks_t = attnp.tile([128, TT, 128], BF, tag="masks", name="masks_t")
            masks = [masks_t[:, t, :] for t in range(TT)]

            # ======== phase 1: LN1 + transpose to feature-major fp8 ========
            with (
                pool("ph1", bufs=1) as ph1p,
                pool("ph1s", bufs=6) as ph1s,
                pool("ph1n", bufs=4) as ph1n,
                pool("wqkv", bufs=1) as wp_,
            ):
                xnT8 = [ph1p.tile([128, 2, T], F8, tag=f"xnT{t}", name=f"xnT{t}") for t in range(KT)]
                xnTq8 = [ph1p.tile([128, 2, NQ], F8, tag=f"xnTq{t}", name=f"xnTq{t}") for t in range(KT)]
                wq8_t = wp_.tile([128, KT, 2, C], F8, tag="wq8", name="wq8")
                wk8_t = wp_.tile([128, KT, 2, C], F8, tag="wk8", name="wk8")
                wv8_t = wp_.tile([128, KT, 2, C], F8, tag="wv8", name="wv8")
                wq8 = [wq8_t[:, t, :, :] for t in range(KT)]
                wk8 = [wk8_t[:, t, :, :] for t in range(KT)]
                wv8 = [wv8_t[:, t, :, :] for t in range(KT)]
                xts = {}

                def x_dma(tt, eng):
                    xts[tt] = ph1s.tile([128, C], BF, tag="xt", name="xt")
                    eng.dma_start(xts[tt][:], d["d_x"][tt * 128 : (tt + 1) * 128, :])

                # x tiles split across the sync and gpsimd queues (halves the
                # per-queue issue seriality; buffer-reuse waits on one queue
                # can't starve the other); QKV weights early on sync; big
                # late-phase weights from the scalar stream mid-phase-1.
                for tt in range(4):
                    x_dma(tt, nc.sync)
                nc.sync.dma_start(wk8_t[:], d["d_wk"][:].rearrange("k p r c -> p k r c"))
                for tt in range(4, 16):
                    x_dma(tt, nc.sync)
                nc.sync.dma_start(wq8_t[:], d["d_wq"][:].rearrange("k p r c -> p k r c"))
                nc.sync.dma_start(wv8_t[:], d["d_wv"][:].rearrange("k p r c -> p k r c"))
                nc.sync.dma_start(masks_t[:], d["d_masks"][:].rearrange("k p c -> p k c"))
                nc.gpsimd.dma_start(xo_t[:], d["d_xob"][:].rearrange("(t p) c -> p t c", p=128))

                with (
                    pool("ph1t", bufs=2, space="PSUM") as ph1t,
                    pool("ph2k", bufs=2, space="PSUM") as ph2k,
                ):
                    # PE warm-up: ramp the p-state while the first LayerNorm
                    # group is still in flight
                    warm = ph2k.tile([128, 512], F32, tag="pqk", name="warm")
                    for _ in range(WARM):
                        nc.tensor.matmul(warm[:, 0:128], ident[:], ident[:])
                    for ttg in range(4):
                        if ttg == 1:
                            # big late-phase weights: issued from the scalar
                            # stream once the early x tiles are in flight
                            nc.scalar.dma_start(wp_t[:], d["d_wp"][:].rearrange("(t p) c -> p t c", p=128))
                            nc.scalar.dma_start(wfc_t[:], d["d_wfc"][:].rearrange("(t p) c -> p t c", p=128))
                            nc.scalar.dma_start(
                                wfc2A[:],
                                d["d_wfc2"][: FT // 2 * 128, :].rearrange("(t p) c -> p t c", p=128),
                            )
                        mvb = small.tile([128, 4, 2], F32, tag="mvb", name="mvb")
                        for ti in range(4):
                            tt = ttg * 4 + ti
                            stats = small.tile([128, 3, 6], F32, tag="bnstats", name="bnstats")
                            xg = xts[tt][:].rearrange("p (a b) -> p a b", b=256)
                            for a in range(3):
                                nc.vector.bn_stats(out=stats[:, a, :], in_=xg[:, a, :])
                            nc.vector.bn_aggr(out=mvb[:, ti, :], in_=stats[:])
                        # rstd for the 4 tiles of this group in one NR batch
                        # (eps is negligible against var ~ 1)
                        rstd4 = _nr_rsqrt(nc, small, mvb[:, :, 1], 4, iters=2)
                        ptb = ph1t.tile([128, CT, 512], BF, tag="ptb", name="ptb")
                        for ti in range(4):
                            tt = ttg * 4 + ti
                            xn = ph1n.tile([128, C], BF, tag="xn", name="xn")
                            nc.vector.tensor_scalar(
                                out=xn[:], in0=xts[tt][:],
                                scalar1=mvb[:, ti, 0:1], scalar2=rstd4[:, ti : ti + 1],
                                op0=ALU.subtract, op1=ALU.mult,
                            )
                            for ct in range(CT):
                                nc.tensor.transpose(
                                    ptb[:, ct, ti * 128 : (ti + 1) * 128],
                                    xn[:, ct * 128 : (ct + 1) * 128], ident[:],
                                )
                        # fp8 conversion: pure casts on the scalar engine,
                        # one ACT per DoubleRow pair
                        for kt in range(KT):
                            nc.scalar.activation(
                                out=xnT8[kt][:, :, ttg * 512 : (ttg + 1) * 512],
                                in_=ptb[:, 2 * kt : 2 * kt + 2, :], func=AF.Identity,
                            )
                            # own q-tile of this group sits at ti == 3
                            nc.scalar.activation(
                                out=xnTq8[kt][:, :, ttg * 128 : (ttg + 1) * 128],
                                in_=ptb[:, 2 * kt : 2 * kt + 2, 384:512],
                                func=AF.Identity,
                            )
                        # kT chunk ttg depends only on this token-group
                        for f in range(CT):
                            ps = ph2k.tile([128, 512], F32, tag="pqk", name="pk")
                            for kt in range(KT):
                                nc.tensor.matmul(
                                    ps[:], wk8[kt][:, :, f * 128 : (f + 1) * 128],
                                    xnT8[kt][:, :, ttg * 512 : (ttg + 1) * 512],
                                    start=(kt == 0), stop=(kt == KT - 1),
                                    perf_mode=DR,
                                )
                            dstk = kT[f][:, ttg * 512 : (ttg + 1) * 512]
                            if f < 4:
                                nc.scalar.activation(
                                    out=dstk, in_=ps[:], func=AF.Identity,
                                    bias=bk[:, f : f + 1],
                                )
                            else:
                                nc.vector.tensor_scalar(
                                    out=dstk, in0=ps[:], scalar1=bk[:, f : f + 1],
                                    scalar2=None, op0=ALU.add,
                                )

                # ======== phase 2: V (with bias-row), Q^T (fp8 DoubleRow) ====
                with (
                    pool("ph2ps", bufs=2, space="PSUM") as ph2ps,
                    pool("ph2pv", bufs=2, space="PSUM") as ph2pv,
                ):
                    def v_tiles(lo_t, hi_t):
                        for tt in range(lo_t, hi_t):
                            pv = ph2pv.tile([128, C], F32, tag="pv", name="pv")
                            for lo, hi in ((0, 512), (512, 768)):
                                for kt in range(KT):
                                    nc.tensor.matmul(
                                        pv[:, lo:hi],
                                        xnT8[kt][:, :, tt * 128 : (tt + 1) * 128],
                                        wv8[kt][:, :, lo:hi],
                                        start=(kt == 0), stop=(kt == KT - 1),
                                        perf_mode=DR,
                                    )
                            vt = V[tt][:].rearrange("p (h e) -> p h e", e=HD + 32)
                            pvh = pv[:].rearrange("p (h e) -> p h e", e=HD)
                            nc.gpsimd.memset(vt[:, :, HD : HD + 32], WS)
                            nc.vector.tensor_copy(vt[:, :, 0:HD], pvh[:])
                    # V tiles 0..11 (needed from attention kp=0/4/8)
                    v_tiles(0, 12)
                    # qT[f] [128, 512] = (Wq[:, f].T @ xnT[own q tiles]) + bq
                    for f in range(CT):
                        ps = ph2ps.tile([128, NQ], F32, tag="pq", name="pq")
                        for kt in range(KT):
                            nc.tensor.matmul(
                                ps[:], wq8[kt][:, :, f * 128 : (f + 1) * 128],
                                xnTq8[kt][:], start=(kt == 0), stop=(kt == KT - 1),
                                perf_mode=DR,
                            )
                        if f % 2 == 0:
                            nc.scalar.activation(
                                out=qT[f][:], in_=ps[:], func=AF.Identity,
                                bias=bq[:, f : f + 1],
                            )
                        else:
                            nc.vector.tensor_scalar(
                                out=qT[f][:], in0=ps[:], scalar1=bq[:, f : f + 1],
                                scalar2=None, op0=ALU.add,
                            )
                    # V tiles 12..15 are only read at attention kp>=12
                    v_tiles(12, 16)

            # ======== phase 3: attention (bf16, 4 heads in flight) ========
            # exp is batched over head PAIRS (one ACT op per pair) since ACT
            # runs 1x with a ~352-cycle fixed cost per instruction.
            with (
                pool("ph3", bufs=8) as ph3s,
                pool("ph3ps", bufs=2, space="PSUM") as ph3ps,
                pool("ph3pa", bufs=1, space="PSUM") as ph3pa,
            ):
                for hg in range(H // 4):
                    hs = [hg * 4 + i for i in range(4)]
                    pavs = {
                        h: ph3pa.tile(
                            [128, NQ], F32, tag=f"pav{h % 4}", name=f"pav{h % 4}"
                        )
                        for h in hs
                    }
                    for kp in range(TT):
                        cs = 128 * (kp // 4)
                        psbs = {}
                        for pi in range(2):
                            hA, hB = hs[2 * pi], hs[2 * pi + 1]
                            ps2 = ph3ps.tile([128, 2, NQ], F32, tag="ps2", name="ps2")
                            for r, h in ((0, hA), (1, hB)):
                                ro = (h % 2) * 64
                                nc.tensor.matmul(
                                    ps2[:, r, cs:NQ],
                                    kT[h // 2][ro : ro + 64, kp * 128 : (kp + 1) * 128],
                                    qT[h // 2][ro : ro + 64, cs:NQ],
                                )
                            p_sb = ph3s.tile([128, 2, NQ], BF, tag="p_sb", name="p_sb")
                            # q,k both carry the 32x fp8 weight scale
                            nc.scalar.activation(
                                out=p_sb[:, :, cs:NQ], in_=ps2[:, :, cs:NQ],
                                func=AF.Exp, scale=0.125 / (WS * WS),
                            )
                            # only the first in-suffix 128-col block is ever
                            # not all-ones (across every core layout)
                            for r, h in ((0, hA), (1, hB)):
                                nc.vector.tensor_mul(
                                    p_sb[:, r, cs : cs + 128],
                                    p_sb[:, r, cs : cs + 128], masks[kp][:],
                                )
                                psbs[h] = (p_sb, r)
                        for h in hs:
                            p_sb, r = psbs[h]
                            nc.tensor.matmul(
                                pavs[h][0 : HD + 32, cs:NQ],
                                V[kp][:, h * (HD + 32) : (h + 1) * (HD + 32)],
                                p_sb[:, r, cs:NQ],
                                start=(kp == 0), stop=(kp == TT - 1),
                                skip_group_check=True,
                            )
                    # gather the 4 heads' denominators and invert them in a
                    # single wide fast-reciprocal
                    denall = ph3s.tile([128, NQ], F32, tag="denall", name="denall")
                    for i, h in enumerate(hs):
                        nc.vector.tensor_copy(
                            denall[32 * i : 32 * (i + 1), :],
                            pavs[h][HD : HD + 32, :],
                        )
                    rball = ph3s.tile([128, NQ], F32, tag="rball", name="rball")
                    nc.vector.reciprocal_approx_fast(out=rball[:], in_=denall[:])
                    for i, h in enumerate(hs):
                        ro = (h % 2) * 64
                        for half in range(2):
                            nc.vector.tensor_tensor(
                                out=yT[h // 2][ro + 32 * half : ro + 32 * (half + 1), :],
                                in0=pavs[h][32 * half : 32 * (half + 1), :],
                                in1=rball[32 * i : 32 * (i + 1), :], op=ALU.mult,
                            )

        # ======== phase 4: proj + residual + LN2 ========
        with pool("mlp_sb", bufs=1) as mlpp:
            x2_t = mlpp.tile([128, 4, C], F32, tag="x2", name="x2_t")
            x2 = [x2_t[:, j, :] for j in range(4)]
            x2b_t = mlpp.tile([128, 4, C], F32, tag="x2b", name="x2b_t")
            x2b = [x2b_t[:, j, :] for j in range(4)]
            xn2_t = mlpp.tile([128, CT, NQ], BF, tag="xn2T", name="xn2_t")
            xn2T = [xn2_t[:, t, :] for t in range(CT)]
            hT_t = mlpp.tile([128, FT, NQ], BF, tag="hT", name="hT_t")
            hT = [hT_t[:, t, :] for t in range(FT)]
            wfc2B = mlpp.tile([128, FT // 2, C], BF, tag="wfc2B", name="wfc2B")
            nc.sync.dma_start(
                wfc2B[:],
                d["d_wfc2"][FT // 2 * 128 :, :].rearrange("(t p) c -> p t c", p=128),
            )
            wfc2 = [wfc2A[:, t, :] for t in range(FT // 2)] + [
                wfc2B[:, t, :] for t in range(FT // 2)
            ]
            with (
                pool("mlp1s", bufs=3) as m1s,
                pool("ph4p", bufs=2, space="PSUM") as ph4p,
                pool("ph4t", bufs=4, space="PSUM") as ph4t,
            ):
                for qt in range(4):
                    pp = ph4p.tile([128, C], F32, tag="pp", name="pp")
                    for lo, hi in ((0, 512), (512, 768)):
                        for ct in range(CT):
                            nc.tensor.matmul(
                                pp[:, lo:hi],
                                yT[ct][:, qt * 128 : (qt + 1) * 128],
                                wp[ct][:, lo:hi],
                                start=(ct == 0), stop=(ct == CT - 1),
                            )
                    nc.vector.tensor_add(x2[qt][:], pp[:], x_own[qt][:])
                    stats = small.tile([128, 3, 6], F32, tag="bnstats", name="bnstats")
                    xg = x2[qt][:].rearrange("p (a b) -> p a b", b=256)
                    for a in range(3):
                        nc.vector.bn_stats(out=stats[:, a, :], in_=xg[:, a, :])
                    mv = small.tile([128, 2], F32, tag="bnaggr", name="bnaggr")
                    nc.vector.bn_aggr(out=mv[:], in_=stats[:])
                    rstd = _nr_rsqrt(nc, small, mv[:, 1:2], 1, iters=2)
                    xn2 = m1s.tile([128, C], BF, tag="xn2", name="xn2")
                    nc.vector.tensor_scalar(
                        out=xn2[:], in0=x2[qt][:], scalar1=mv[:, 0:1], scalar2=rstd[:],
                        op0=ALU.subtract, op1=ALU.mult,
                    )
                    for ct in range(CT):
                        pt = ph4t.tile([128, 128], BF, tag="pt4", name="pt4")
                        nc.tensor.transpose(
                            pt[:], xn2[:, ct * 128 : (ct + 1) * 128], ident[:]
                        )
                        nc.scalar.activation(
                            out=xn2_t[:, ct, qt * 128 : (qt + 1) * 128],
                            in_=pt[:], func=AF.Identity,
                        )

            # ======== phase 5: fc -> hT directly (feature-major out), gelu
            # bias is per-partition; fc2 for query tiles 0/1 streams along as
            # each hT f-tile lands, so fc2 overlaps fc instead of following it
            with pool("po01", bufs=1, space="PSUM") as po01p:
                poA = [
                    po01p.tile([128, C], F32, tag=f"poA{q}", name=f"poA{q}")
                    for q in range(2)
                ]
                with pool("ph5p", bufs=2, space="PSUM") as ph5p:
                    for f in range(FT):
                        ph_ = ph5p.tile([128, 512], F32, tag="ph5", name="ph5")
                        for ct in range(CT):
                            nc.tensor.matmul(
                                ph_[:],
                                wfc[ct][:, f * 128 : (f + 1) * 128],
                                xn2T[ct][:],
                                start=(ct == 0), stop=(ct == CT - 1),
                            )
                        nc.scalar.activation(
                            out=hT[f][:], in_=ph_[:],
                            func=AF.Gelu_apprx_tanh, bias=bfcc[:, f : f + 1],
                        )
                        if f < 4:
                            nc.vector.tensor_add(x2b[f][:], x2[f][:], bfc2_bc[:])
                        for qi in range(2):
                            for lo, hi in ((0, 512), (512, 768)):
                                nc.tensor.matmul(
                                    poA[qi][:, lo:hi],
                                    hT[f][:, qi * 128 : (qi + 1) * 128],
                                    wfc2[f][:, lo:hi],
                                    start=(f == 0), stop=(f == FT - 1),
                                    skip_group_check=True,
                                )

                # ======== phase 7: fc2 for query tiles 2/3 + residual + out ==
                with (
                    pool("mlp2s", bufs=3) as m2s,
                    pool("ph7p", bufs=2, space="PSUM") as ph7p,
                ):
                    for qt in range(4):
                        if qt < 2:
                            po = poA[qt]
                        else:
                            po = ph7p.tile([128, C], F32, tag="po", name="po")
                            for lo, hi in ((0, 512), (512, 768)):
                                for kt in range(FT):
                                    nc.tensor.matmul(
                                        po[:, lo:hi],
                                        hT[kt][:, qt * 128 : (qt + 1) * 128],
                                        wfc2[kt][:, lo:hi],
                                        start=(kt == 0), stop=(kt == FT - 1),
                                    )
                        ot = m2s.tile([128, C], F32, tag="ot", name="ot")
                        nc.vector.tensor_add(ot[:], po[:], x2b[qt][:])
                        nc.sync.dma_start(
                            d["d_out"][qt * 128 : (qt + 1) * 128, :], ot[:]
                        )


# ---------------------------------------------------------------------------
# Host-side wrapper
# ---------------------------------------------------------------------------
_PROGRAM = None


def _get_program():
    global _PROGRAM
    if _PROGRAM is None:
        _PROGRAM = build_program()
    return _PROGRAM


def make_in_maps(x, ln1_g, ln1_b, W_attn, b_attn, W_proj, b_proj,
                 ln2_g, ln2_b, W_fc, b_fc, W_fc2, b_fc2):
    x = np.asarray(x, np.float32)
    # fold LN gains/biases into the downstream weights (host-side, free)
    g1 = np.asarray(ln1_g, np.float32)
    b1 = np.asarray(ln1_b, np.float32)
    g2 = np.asarray(ln2_g, np.float32)
    b2 = np.asarray(ln2_b, np.float32)
    Wa = np.asarray(W_attn, np.float32)
    Wag = Wa * g1[:, None]
    ba = np.asarray(b_attn, np.float32) + b1 @ Wa
    Wf = np.asarray(W_fc, np.float32)
    Wfg = Wf * g2[:, None]
    bf = np.asarray(b_fc, np.float32) + b2 @ Wf

    shared = {
        "wq": pack_dr(Wag[:, 0:C]),
        "wk": pack_dr(Wag[:, C : 2 * C]),
        "wv": pack_dr(Wag[:, 2 * C : 3 * C]),
        "wp": np.asarray(W_proj, BF16),
        "wfc": np.asarray(Wfg, BF16),
        "wfc2": np.asarray(W_fc2, BF16),
        # [bq bk] ride the 32x weight scale
        "consts2": np.concatenate([
            (ba[0:C] * WS).reshape(CT, 128).T,
            (ba[C : 2 * C] * WS).reshape(CT, 128).T,
        ], axis=1).copy(),
        "constsb": np.broadcast_to(
            np.asarray(b_fc2, np.float32), (128, C)).copy(),
        "bfc2d": np.ascontiguousarray(bf.reshape(FT, 128).T),
        "ident": np.eye(128, dtype=BF16),
    }
    bp = np.asarray(b_proj, np.float32)
    in_maps, layouts = [], []
    for core in range(8):
        b, g = core // 4, core % 4
        qtiles, perm = core_layout(g)
        idx = np.concatenate([np.arange(t * 128, (t + 1) * 128) for t in perm])
        own = np.concatenate([np.arange(t * 128, (t + 1) * 128) for t in qtiles])
        m = dict(shared)
        m["x_perm"] = np.ascontiguousarray(x[b][idx]).astype(BF16)
        m["x_own_b"] = np.ascontiguousarray(x[b][own] + bp)
        m["masks"] = core_masks(qtiles, perm)
        in_maps.append(m)
        layouts.append((b, own))
    return in_maps, layouts


def unshard(results, layouts):
    out = np.empty((B, T, C), np.float32)
    for r, (b, own) in zip(results, layouts):
        out[b][own] = r["out"]
    return out


def kernel(**inputs):
    from concourse.bass_utils import run_bass_kernel_spmd

    nc = _get_program()
    in_maps, layouts = make_in_maps(**inputs)
    res = run_bass_kernel_spmd(nc, in_maps, core_ids=list(range(8)))
    return unshard(res.results, layouts)
